# revision 22
# baseline (speedup 1.0000x reference)
"""Trainium2 Bass kernel for nn_ActionPredictionModel (scatter_memory).

Data-parallel over graphs: 8 graphs (72 nodes) per NeuronCore, weights
replicated (collectives measured ~80us launch-skew here, so none used).
Per core:
  - spec MLP layer 1 with W1 as the *moving* operand (b-major output):
    fp32 PE cost is ~2.2ns/col of moving stream, vs ~3.7ns/col as
    stationary, so stationary is the tiny spec tile (8 cols) and all of
    W1 streams through as rhs. b1 is folded in via a constant-1 spec row.
    h1 [8, 904] is then relu'd and PE-transposed (8x [8,113]->[113,8])
    for the hid-major layer 2.
  - value head (sum-pool readout + spec -> scalar)
  - pair action features; block-diagonal structure: only the 9x9
    same-graph pair blocks are materialized ([128ch, 648pairs])
  - per-graph flatten + indexmask gather (gpsimd ap_gather, ucode
    warmed early) + softmax (DVE + one ACT Exp, table warmed early)
Host does only sharding/layout marshalling (transpose, pad, tile-pack,
index remap to the on-device fp layout) and output concatenation.
"""

import numpy as np

# problem dims (hardcoded per contract)
B, NPG, H = 64, 9, 128
SL, SC, BOND, ASL = 1801, 100, 3, 243
NCORES = 8
BPC = B // NCORES            # graphs per core = 8
NODES = BPC * NPG            # nodes per core = 72
PAIRS = BPC * NPG * NPG      # same-graph pairs per core = 648

KT = 15                      # k-tiles over spec dim (14*128 + 10(incl bias row))
HID = 900
HIDP = 904                   # padded hidden (8 * 113)
MCH = HIDP // 8              # hid chunk for transposes / L2 = 113
HHALF = HIDP // 2            # 452 (psum bank-sized moving chunks)

# consts column offsets
OFF_WA2A, OFF_WA2B, OFF_WV1, OFF_WA2C = 0, 128, 256, 384
OFF_WV2, OFF_B2, OFF_BV1, OFF_BV2, OFF_BA2, OFF_BF = 512, 513, 514, 515, 516, 517
OFF_WF, OFF_EYE, OFF_W2T = 518, 521, 529
OFF_WFD, OFF_BFC, OFF_SUMS = 1329, 1457, 1458
CF = 1458 + 128

_CACHE = {}
DEBUG_TAPS = False


def _f32(x):
    return np.ascontiguousarray(np.asarray(x), dtype=np.float32)


def _build_nc():
    import concourse.mybir as mybir
    import concourse.tile as tile
    import concourse.bacc as bacc
    import concourse.bass as bass

    f32 = mybir.dt.float32
    i16 = mybir.dt.int16
    Alu = mybir.AluOpType
    Act = mybir.ActivationFunctionType

    nc = bacc.Bacc("TRN2", target_bir_lowering=False, debug=False, num_devices=1)

    consts_d = nc.declare_dram_parameter("consts", [128, CF], f32, isOutput=False)
    acts_d = nc.declare_dram_parameter("acts", [128, KT * BPC + NODES], f32, isOutput=False)
    w1k_d = [nc.declare_dram_parameter(f"w1k{k}", [128 if k < KT - 1 else 10, HIDP], f32, isOutput=False)
             for k in range(KT)]
    mask_d = nc.declare_dram_parameter("mask8", [BPC, ASL], f32, isOutput=False)
    idx_d = nc.declare_dram_parameter("idx16", [128, 16], i16, isOutput=False)
    selm_d = nc.declare_dram_parameter("selm", [128, 256], f32, isOutput=False)
    outp_d = nc.declare_dram_parameter("out_p", [128, ASL], f32, isOutput=True)
    outv_d = nc.declare_dram_parameter("out_v", [1, BPC], f32, isOutput=True)

    with tile.TileContext(nc) as tc:
        with (
            tc.tile_pool(name="cpool", bufs=1) as cpool,
            tc.tile_pool(name="pab", bufs=1, space="PSUM") as pab,
            tc.tile_pool(name="psh", bufs=2, space="PSUM") as psh,
            tc.tile_pool(name="ph1", bufs=2, space="PSUM") as ph1,
            tc.tile_pool(name="ptr", bufs=2, space="PSUM") as ptr,
        ):
            # ---- input loads: acts + per-k W1 chunks gate the stream ----
            acts = cpool.tile([128, KT * BPC + NODES], f32)
            nc.scalar.dma_start(acts[:], acts_d[:])
            idxs = cpool.tile([128, 16], i16)
            nc.sync.dma_start(idxs[:], idx_d[:])
            w1ts = []
            for k in range(KT):
                kk = 128 if k < KT - 1 else 10
                wt = cpool.tile([kk, HIDP], f32, tag=f"w1k{k}")
                eng = nc.sync if k % 2 == 0 else nc.scalar
                eng.dma_start(wt[:], w1k_d[k][:])
                w1ts.append(wt)
            consts = cpool.tile([128, CF], f32)
            nc.sync.dma_start(consts[:], consts_d[:])

            # mask tile (memset first: only rows 16*b are real)
            Mt = cpool.tile([128, ASL], f32, tag="Mt")
            nc.vector.memset(Mt[:], 0.0)
            selm = cpool.tile([128, 256], f32)
            nc.scalar.dma_start(selm[:], selm_d[:])
            m_out = bass.AP(Mt[:].tensor, Mt[:].offset, [[16 * ASL, BPC], [1, ASL]])
            nc.scalar.dma_start(m_out, mask_d[:])

            # warm-ups: ACT Exp table + gpsimd ap_gather ucode (hide under stream)
            warm = cpool.tile([1, 1], f32)
            nc.vector.memset(warm[:], 0.0)
            warmo = cpool.tile([1, 1], f32)
            nc.scalar.activation(warmo[:], warm[:], Act.Exp)
            gwi = cpool.tile([16, 4], f32)
            gwx = cpool.tile([16, 1], i16)
            gwo = cpool.tile([16, 16], f32)
            nc.vector.memset(gwi[:], 0.0)
            nc.vector.memset(gwx[:], 0)
            nc.gpsimd.ap_gather(gwo[:], gwi[:], gwx[:], channels=16, num_elems=4, d=1, num_idxs=16)

            sp = acts[:, 0 : KT * BPC]               # spT own graphs [128, 15*8]
            nf = acts[:, KT * BPC : KT * BPC + NODES]  # nfT own graphs [128, 72]

            # ---- spec MLP layer 1: b-major, W1 moving, accumulate in PSUM ----
            h1bs = cpool.tile([BPC, HIDP], f32)
            if True:
                h1p0 = ph1.tile([BPC, HHALF], f32, tag="h1b")
                h1p1 = ph1.tile([BPC, HHALF], f32, tag="h1b")
                for k in range(KT):
                    kk = 128 if k < KT - 1 else 10  # last tile: 9 spec rows + bias row
                    wsl = w1ts[k][:]
                    lhs = sp[:kk, BPC * k : BPC * (k + 1)]
                    nc.tensor.matmul(h1p0[:], lhs, wsl[:, :HHALF],
                                     start=(k == 0), stop=(k == KT - 1))
                    nc.tensor.matmul(h1p1[:], lhs, wsl[:, HHALF:],
                                     start=(k == 0), stop=(k == KT - 1))
                # relu (bias already folded via the constant-1 spec row)
                nc.vector.tensor_scalar_max(h1bs[:, :HHALF], h1p0[:], 0.0)
                nc.vector.tensor_scalar_max(h1bs[:, HHALF:], h1p1[:], 0.0)

            if True:
                # ---- transpose h1 to hid-major: 8x PE transpose [8,113]->[113,8] ----
                eye = consts[:BPC, OFF_EYE : OFF_EYE + BPC]
                h1ts = cpool.tile([MCH, 64], f32)
                for j in range(8):
                    tp = ptr.tile([MCH, BPC], f32, tag="tr")
                    nc.tensor.transpose(tp[:], h1bs[:, MCH * j : MCH * (j + 1)], eye)
                    nc.vector.tensor_copy(h1ts[:, BPC * j : BPC * (j + 1)], tp[:])

                # ---- layer 2: sT[q, b] accumulated over 8 hid chunks ----
                sps = psh.tile([SC, BPC], f32, tag="sh")
                for j in range(8):
                    nc.tensor.matmul(
                        sps[:],
                        consts[:MCH, OFF_W2T + SC * j : OFF_W2T + SC * (j + 1)],
                        h1ts[:, BPC * j : BPC * (j + 1)],
                        start=(j == 0), stop=(j == 7),
                    )
                sTs = cpool.tile([SC + 1, BPC], f32)
                nc.vector.memset(sTs[:], 1.0)
                nc.vector.tensor_scalar(sTs[:SC, :], sps[:], consts[:SC, OFF_B2 : OFF_B2 + 1],
                                        0.0, op0=Alu.add, op1=Alu.max)

                # ---- value head ----
                ro = cpool.tile([128, BPC], f32)
                nc.vector.reduce_sum(ro[:], nf.rearrange("p (b n) -> p b n", n=NPG),
                                     axis=mybir.AxisListType.X)
                y1 = psh.tile([64, BPC], f32, tag="sh")
                nc.tensor.matmul(y1[:], consts[:, OFF_WV1 : OFF_WV1 + 64], ro[:], start=True, stop=False)
                nc.tensor.matmul(y1[:], consts[:SC, OFF_WV1 + 64 : OFF_WV1 + 128], sTs[:SC, :], start=False, stop=True)
                y1s = cpool.tile([64, BPC], f32)
                nc.vector.tensor_scalar(y1s[:], y1[:], consts[:64, OFF_BV1 : OFF_BV1 + 1],
                                        0.0, op0=Alu.add, op1=Alu.max)
                vps = psh.tile([1, BPC], f32, tag="sh")
                nc.tensor.matmul(vps[:], consts[:64, OFF_WV2 : OFF_WV2 + 1], y1s[:], start=True, stop=True)
                vs = cpool.tile([1, BPC], f32)
                nc.vector.tensor_scalar_add(vs[:], vps[:], consts[:1, OFF_BV2 : OFF_BV2 + 1])
                nc.scalar.dma_start(outv_d[:], vs[:])

                # ---- pair features: hT[c, (b,i,j)] ----
                nfr = cpool.tile([128, NODES], f32)
                nc.vector.tensor_scalar_max(nfr[:], nf, 0.0)
                aips = pab.tile([128, NODES], f32, tag="aips")
                nc.tensor.matmul(aips[:], consts[:, OFF_WA2A : OFF_WA2A + 128], nfr[:], start=True, stop=True)
                bjps = pab.tile([128, NODES], f32, tag="bjps")
                nc.tensor.matmul(bjps[:], consts[:, OFF_WA2B : OFF_WA2B + 128], nfr[:], start=True, stop=True)
                bjs = cpool.tile([128, NODES], f32)
                nc.vector.tensor_copy(bjs[:], bjps[:])
                dps = psh.tile([128, BPC], f32, tag="sh")
                nc.tensor.matmul(dps[:], consts[: SC + 1, OFF_WA2C : OFF_WA2C + 128], sTs[:], start=True, stop=True)
                dt2 = cpool.tile([128, BPC], f32)
                nc.vector.tensor_copy(dt2[:], dps[:])
                ai2 = cpool.tile([128, NODES], f32)
                nc.vector.tensor_tensor(
                    ai2[:].rearrange("p (b i) -> p b i", i=NPG),
                    aips[:].rearrange("p (b i) -> p b i", i=NPG),
                    dt2[:].unsqueeze(2).broadcast_to([128, BPC, NPG]),
                    op=Alu.add,
                )
                hT = cpool.tile([128, PAIRS], f32)
                nc.vector.tensor_tensor(
                    hT[:].rearrange("p (b i j) -> p b i j", i=NPG, j=NPG),
                    ai2[:].rearrange("p (b i) -> p b i", i=NPG).unsqueeze(3).broadcast_to([128, BPC, NPG, NPG]),
                    bjs[:].rearrange("p (b j) -> p b j", j=NPG).unsqueeze(2).broadcast_to([128, BPC, NPG, NPG]),
                    op=Alu.add,
                )
                nc.vector.tensor_scalar_max(hT[:], hT[:], 0.0)

                # ---- saf planes, group-partition layout ----
                # X0[16b+t, pair] = sum_c WfD[c, 16b+t] * hT[c, pair] = saf
                # bond-plane t of every pair (WfD col 16b+t = Wf[:, t]).
                # Rows 16b+3..16b+15 are zero. Then + bf (per-partition col).
                xp1 = ptr.tile([128, PAIRS // 2], f32, tag="tr")
                xp2 = ptr.tile([128, PAIRS // 2], f32, tag="tr")
                nc.tensor.matmul(xp1[:], consts[:, OFF_WFD : OFF_WFD + 128], hT[:, : PAIRS // 2], start=True, stop=True)
                nc.tensor.matmul(xp2[:], consts[:, OFF_WFD : OFF_WFD + 128], hT[:, PAIRS // 2 :], start=True, stop=True)
                x0 = cpool.tile([128, PAIRS], f32)
                nc.vector.tensor_scalar_add(x0[:, : PAIRS // 2], xp1[:], consts[:, OFF_BFC : OFF_BFC + 1])
                nc.vector.tensor_scalar_add(x0[:, PAIRS // 2 :], xp2[:], consts[:, OFF_BFC : OFF_BFC + 1])

                # ---- gather (pair index per group) + bond select + sum ----
                G3 = cpool.tile([128, 256], f32)
                nc.gpsimd.ap_gather(G3[:], x0[:], idxs[:], channels=128, num_elems=PAIRS, d=1, num_idxs=256)
                P = cpool.tile([128, 256], f32)
                nc.vector.tensor_tensor(P[:], G3[:], selm[:], op=Alu.mult)
                Gp = ptr.tile([128, 256], f32, tag="tr")
                nc.tensor.matmul(Gp[:], consts[:, OFF_SUMS : OFF_SUMS + 128], P[:], start=True, stop=True)
                X2 = cpool.tile([128, ASL], f32)
                nc.vector.tensor_tensor(X2[:], Gp[:, :ASL], Mt[:], op=Alu.add)
                nmx = cpool.tile([128, 1], f32)
                nc.vector.reduce_max(nmx[:], X2[:], axis=mybir.AxisListType.X, negate=True)
                E = cpool.tile([128, ASL], f32)
                sums = cpool.tile([128, 1], f32)
                nc.scalar.activation(E[:], X2[:], Act.Exp, bias=nmx[:], accum_out=sums[:])
                rc = cpool.tile([128, 1], f32)
                nc.vector.reciprocal(rc[:], sums[:])
                OU = cpool.tile([128, ASL], f32)
                nc.vector.tensor_scalar_mul(OU[:], E[:], rc[:])
                nc.sync.dma_start(outp_d[:], OU[:])

                if DEBUG_TAPS:
                    taps = {
                        "t_h1bs": h1bs, "t_h1ts": h1ts, "t_sTs": sTs, "t_ro": ro,
                        "t_y1s": y1s, "t_nfr": nfr, "t_ai2": ai2,
                        "t_bjs": bjs, "t_hT": hT, "t_x0": x0,
                        "t_G3": G3, "t_P": P, "t_X2": X2, "t_sums": sums,
                    }
                    for tname, ttile in taps.items():
                        shp = list(ttile[:].shape)
                        td = nc.declare_dram_parameter(tname, shp, f32, isOutput=True)
                        nc.sync.dma_start(td[:], ttile[:])

    nc.compile()
    return nc


def _marshal(node_features, specs, mask, indexmask, W1, b1, W2, b2,
             Wv1, bv1, Wv2, bv2, Wa2, ba2, Wf, bf):
    """Host-side sharding + layout packing. Returns in_maps (one per core)."""
    # W1 with b1 folded as an extra (constant-1-input) row, k-tile packed
    w1p = np.zeros((1802, HIDP), np.float32)
    w1p[:SL, :HID] = W1
    w1p[SL, :HID] = b1
    w1ks = {f"w1k{k}": np.ascontiguousarray(w1p[128 * k : 128 * k + (128 if k < KT - 1 else 10)])
            for k in range(KT)}
    w2p = np.zeros((HIDP, SC), np.float32)
    w2p[:HID] = W2

    consts = np.zeros((128, CF), np.float32)
    consts[:, OFF_WA2A : OFF_WA2A + 128] = Wa2[0:128]
    consts[:, OFF_WA2B : OFF_WA2B + 128] = Wa2[128:256]
    consts[:, OFF_WV1 : OFF_WV1 + 64] = Wv1[0:128]
    consts[:100, OFF_WV1 + 64 : OFF_WV1 + 128] = Wv1[128:228]
    consts[:100, OFF_WA2C : OFF_WA2C + 128] = Wa2[256:356]
    consts[100, OFF_WA2C : OFF_WA2C + 128] = ba2
    consts[:64, OFF_WV2] = Wv2[:, 0]
    consts[:100, OFF_B2] = b2
    consts[:64, OFF_BV1] = bv1
    consts[:1, OFF_BV2] = bv2
    consts[:, OFF_BA2] = ba2
    consts[:BOND, OFF_BF] = bf
    consts[:, OFF_WF : OFF_WF + BOND] = Wf
    consts[:BPC, OFF_EYE : OFF_EYE + BPC] = np.eye(BPC, dtype=np.float32)
    for b in range(BPC):
        consts[:, OFF_WFD + 16 * b : OFF_WFD + 16 * b + BOND] = Wf
        consts[16 * b : 16 * b + BOND, OFF_BFC] = bf
        for t in range(BOND):
            consts[16 * b + t, OFF_SUMS + 16 * b] = 1.0
    consts[:MCH, OFF_W2T : OFF_W2T + 800] = w2p.reshape(8, MCH, SC).transpose(1, 0, 2).reshape(MCH, 800)

    # index remap: group b gathers pair columns 81*b + v//3 from the plane
    # tile; the bond (v%3) is selected via the SelM one-hot afterwards
    v = indexmask.astype(np.int64)

    in_maps = []
    for c in range(NCORES):
        gsl = slice(c * BPC, (c + 1) * BPC)
        nsl = slice(c * NODES, (c + 1) * NODES)
        # spec transposed + k-tiled + constant-1 bias row (row 1801)
        spc = np.zeros((BPC, KT * 128), np.float32)
        spc[:, :SL] = specs[gsl, 0, :]
        spc[:, SL] = 1.0
        spT = spc.reshape(BPC, KT, 128).transpose(2, 1, 0).reshape(128, KT * BPC)
        acts = np.zeros((128, KT * BPC + NODES), np.float32)
        acts[:, 0 : KT * BPC] = spT
        acts[:, KT * BPC :] = node_features[nsl].T
        vc = v[gsl]                                    # [8, 243]
        padidx = np.zeros((BPC, 256), np.int16)
        padidx[:, :ASL] = (np.arange(BPC)[:, None] * NPG * NPG + vc // BOND).astype(np.int16)
        idx16 = padidx.reshape(BPC, 16, 16).transpose(0, 2, 1).reshape(128, 16)
        selm = np.zeros((128, 256), np.float32)
        for b in range(BPC):
            for t in range(BOND):
                selm[16 * b + t, :ASL] = (vc[b] % BOND == t)
        in_maps.append({
            "consts": consts,
            "acts": acts,
            **w1ks,
            "mask8": np.ascontiguousarray(mask[gsl], np.float32),
            "idx16": np.ascontiguousarray(idx16),
            "selm": selm,
        })
    return in_maps


def _run(inputs, trace=False):
    from concourse.bass_utils import run_bass_kernel_spmd

    if "nc" not in _CACHE:
        _CACHE["nc"] = _build_nc()
    nc = _CACHE["nc"]

    in_maps = _marshal(
        _f32(inputs["node_features"]), _f32(inputs["specs"]),
        _f32(inputs["mask"]), np.asarray(inputs["indexmask"]),
        _f32(inputs["W1"]), _f32(inputs["b1"]), _f32(inputs["W2"]), _f32(inputs["b2"]),
        _f32(inputs["Wv1"]), _f32(inputs["bv1"]), _f32(inputs["Wv2"]), _f32(inputs["bv2"]),
        _f32(inputs["Wa2"]), _f32(inputs["ba2"]), _f32(inputs["Wf"]), _f32(inputs["bf"]),
    )
    res = run_bass_kernel_spmd(nc, in_maps, core_ids=list(range(NCORES)), trace=trace)
    probs = np.concatenate([res.results[c]["out_p"][::16] for c in range(NCORES)], axis=0)
    v = np.concatenate([res.results[c]["out_v"][0] for c in range(NCORES)])[:, None]
    return (probs, v.astype(np.float32)), res


def kernel(**inputs):
    (probs, v), _ = _run(inputs, trace=False)
    return probs, v


# revision 33
# speedup vs baseline: 1.0351x; 1.0351x over previous
"""Trainium2 Bass kernel for nn_ActionPredictionModel (scatter_memory).

Data-parallel over graphs: 8 graphs (72 nodes) per NeuronCore, weights
replicated (collectives measured ~80us launch-skew here, so none used).
Per core:
  - spec MLP layer 1 with W1 as the *moving* operand (b-major output):
    fp32 PE cost is ~2.2ns/col of moving stream, vs ~3.7ns/col as
    stationary, so stationary is the tiny spec tile (8 cols) and all of
    W1 streams through as rhs. b1 is folded in via a constant-1 spec row.
    h1 [8, 904] is then relu'd and PE-transposed (8x [8,113]->[113,8])
    for the hid-major layer 2.
  - value head (sum-pool readout + spec -> scalar)
  - pair action features; block-diagonal structure: only the 9x9
    same-graph pair blocks are materialized ([128ch, 648pairs])
  - per-graph flatten + indexmask gather (gpsimd ap_gather, ucode
    warmed early) + softmax (DVE + one ACT Exp, table warmed early)
Host does only sharding/layout marshalling (transpose, pad, tile-pack,
index remap to the on-device fp layout) and output concatenation.
"""

import numpy as np

# problem dims (hardcoded per contract)
B, NPG, H = 64, 9, 128
SL, SC, BOND, ASL = 1801, 100, 3, 243
NCORES = 8
BPC = B // NCORES            # graphs per core = 8
NODES = BPC * NPG            # nodes per core = 72
PAIRS = BPC * NPG * NPG      # same-graph pairs per core = 648

KT = 15                      # k-tiles over spec dim (14*128 + 10(incl bias row))
HID = 900
HIDP = 904                   # padded hidden (8 * 113)
MCH = HIDP // 8              # hid chunk for transposes / L2 = 113
HHALF = HIDP // 2            # 452 (psum bank-sized moving chunks)

# consts column offsets
OFF_WA2A, OFF_WA2B, OFF_WV1, OFF_WA2C = 0, 128, 256, 384
OFF_WV2, OFF_B2, OFF_BV1, OFF_BV2, OFF_BA2, OFF_BF = 512, 513, 514, 515, 516, 517
OFF_WF, OFF_EYE, OFF_W2T = 518, 521, 529
OFF_WFD, OFF_BFC, OFF_SUMS = 1329, 1457, 1458
CF = 1458 + 128

_CACHE = {}
DEBUG_TAPS = False


def _f32(x):
    return np.ascontiguousarray(np.asarray(x), dtype=np.float32)


def _build_nc():
    import concourse.mybir as mybir
    import concourse.tile as tile
    import concourse.bacc as bacc
    import concourse.bass as bass

    f32 = mybir.dt.float32
    i16 = mybir.dt.int16
    Alu = mybir.AluOpType
    Act = mybir.ActivationFunctionType

    nc = bacc.Bacc("TRN2", target_bir_lowering=False, debug=False, num_devices=1)

    consts_d = nc.declare_dram_parameter("consts", [128, CF], f32, isOutput=False)
    acts_d = nc.declare_dram_parameter("acts", [128, KT * BPC + NODES], f32, isOutput=False)
    w1k_d = [nc.declare_dram_parameter(f"w1k{k}", [128 if k < KT - 1 else 10, HIDP], f32, isOutput=False)
             for k in range(KT)]
    idx_d = nc.declare_dram_parameter("idx16", [128, 16], i16, isOutput=False)
    selm_d = nc.declare_dram_parameter("selm", [128, 256], f32, isOutput=False)
    outp_d = nc.declare_dram_parameter("out_p", [BPC, ASL], f32, isOutput=True)
    outv_d = nc.declare_dram_parameter("out_v", [1, BPC], f32, isOutput=True)

    with tile.TileContext(nc) as tc:
        with (
            tc.tile_pool(name="cpool", bufs=1) as cpool,
            tc.tile_pool(name="pab", bufs=1, space="PSUM") as pab,
            tc.tile_pool(name="psh", bufs=2, space="PSUM") as psh,
            tc.tile_pool(name="ph1", bufs=2, space="PSUM") as ph1,
            tc.tile_pool(name="ptr", bufs=2, space="PSUM") as ptr,
        ):
            # ---- input loads: acts + per-k W1 chunks gate the stream ----
            # (k=14 is tiny -> loads first so the stream starts early)
            acts = cpool.tile([128, KT * BPC + NODES], f32)
            nc.scalar.dma_start(acts[:], acts_d[:])
            w1ts = [None] * KT
            for k in range(KT):
                kk = 128 if k < KT - 1 else 10
                wt = cpool.tile([kk, HIDP], f32, tag=f"w1k{k}")
                eng = nc.sync if k % 2 == 0 else nc.scalar
                eng.dma_start(wt[:], w1k_d[k][:])
                w1ts[k] = wt
            consts = cpool.tile([128, CF], f32)
            nc.sync.dma_start(consts[:], consts_d[:])
            idxs = cpool.tile([128, 16], i16)
            nc.sync.dma_start(idxs[:], idx_d[:])

            # bond-select + mask input (mask rides in selm rows 16b+3)
            selm = cpool.tile([128, 256], f32)
            nc.scalar.dma_start(selm[:], selm_d[:])

            # warm-ups: ACT Exp table + gpsimd ap_gather ucode (hide under stream)
            warm = cpool.tile([1, 1], f32)
            nc.vector.memset(warm[:], 0.0)
            warmo = cpool.tile([1, 1], f32)
            nc.scalar.activation(warmo[:], warm[:], Act.Exp)

            # warm ap_gather: pulls the gpsimd ucode library load off the
            # critical path (the real gather otherwise pays ~8-10us for it)
            gwi = cpool.tile([16, 4], f32)
            gwx = cpool.tile([16, 1], i16)
            gwo = cpool.tile([16, 16], f32)
            nc.vector.memset(gwi[:], 0.0)
            nc.vector.memset(gwx[:], 0)
            nc.gpsimd.ap_gather(gwo[:], gwi[:], gwx[:], channels=16, num_elems=4, d=1, num_idxs=16)

            sp = acts[:, 0 : KT * BPC]               # spT own graphs [128, 15*8]
            nf = acts[:, KT * BPC : KT * BPC + NODES]  # nfT own graphs [128, 72]

            # ---- spec MLP layer 1: b-major, W1 moving, accumulate in PSUM ----
            h1bs = cpool.tile([BPC, HIDP], f32)
            if True:
                h1p0 = ph1.tile([BPC, HHALF], f32, tag="h1b")
                h1p1 = ph1.tile([BPC, HHALF], f32, tag="h1b")
                for k in range(KT):
                    kk = 128 if k < KT - 1 else 10  # last tile: 9 spec rows + bias row
                    wsl = w1ts[k][:]
                    lhs = sp[:kk, BPC * k : BPC * (k + 1)]
                    nc.tensor.matmul(h1p0[:], lhs, wsl[:, :HHALF],
                                     start=(k == 0), stop=(k == KT - 1))
                    nc.tensor.matmul(h1p1[:], lhs, wsl[:, HHALF:],
                                     start=(k == 0), stop=(k == KT - 1))
                # relu (bias already folded via the constant-1 spec row)
                nc.vector.tensor_scalar_max(h1bs[:, :HHALF], h1p0[:], 0.0)
                nc.vector.tensor_scalar_max(h1bs[:, HHALF:], h1p1[:], 0.0)

            if True:
                # ---- transpose h1 to hid-major: 8x PE transpose [8,113]->[113,8] ----
                eye = consts[:BPC, OFF_EYE : OFF_EYE + BPC]
                h1ts = cpool.tile([MCH, 64], f32)
                for j in range(8):
                    tp = ptr.tile([MCH, BPC], f32, tag="tr")
                    nc.tensor.transpose(tp[:], h1bs[:, MCH * j : MCH * (j + 1)], eye)
                    nc.vector.tensor_copy(h1ts[:, BPC * j : BPC * (j + 1)], tp[:])

                # ---- layer 2: sT[q, b] accumulated over 8 hid chunks ----
                sps = psh.tile([SC, BPC], f32, tag="sh")
                for j in range(8):
                    nc.tensor.matmul(
                        sps[:],
                        consts[:MCH, OFF_W2T + SC * j : OFF_W2T + SC * (j + 1)],
                        h1ts[:, BPC * j : BPC * (j + 1)],
                        start=(j == 0), stop=(j == 7),
                    )
                sTs = cpool.tile([SC + 1, BPC], f32)
                nc.vector.memset(sTs[:], 1.0)
                nc.vector.tensor_scalar(sTs[:SC, :], sps[:], consts[:SC, OFF_B2 : OFF_B2 + 1],
                                        0.0, op0=Alu.add, op1=Alu.max)

                # ---- value head ----
                ro = cpool.tile([128, BPC], f32)
                nc.vector.reduce_sum(ro[:], nf.rearrange("p (b n) -> p b n", n=NPG),
                                     axis=mybir.AxisListType.X)
                y1 = psh.tile([64, BPC], f32, tag="sh")
                nc.tensor.matmul(y1[:], consts[:, OFF_WV1 : OFF_WV1 + 64], ro[:], start=True, stop=False)
                nc.tensor.matmul(y1[:], consts[:SC, OFF_WV1 + 64 : OFF_WV1 + 128], sTs[:SC, :], start=False, stop=True)
                y1s = cpool.tile([64, BPC], f32)
                nc.vector.tensor_scalar(y1s[:], y1[:], consts[:64, OFF_BV1 : OFF_BV1 + 1],
                                        0.0, op0=Alu.add, op1=Alu.max)
                vps = psh.tile([1, BPC], f32, tag="sh")
                nc.tensor.matmul(vps[:], consts[:64, OFF_WV2 : OFF_WV2 + 1], y1s[:], start=True, stop=True)
                vs = cpool.tile([1, BPC], f32)
                nc.vector.tensor_scalar_add(vs[:], vps[:], consts[:1, OFF_BV2 : OFF_BV2 + 1])
                nc.scalar.dma_start(outv_d[:], vs[:])

                # ---- pair features: hT[c, (b,i,j)] ----
                nfr = cpool.tile([128, NODES], f32)
                nc.vector.tensor_scalar_max(nfr[:], nf, 0.0)
                aips = pab.tile([128, NODES], f32, tag="aips")
                nc.tensor.matmul(aips[:], consts[:, OFF_WA2A : OFF_WA2A + 128], nfr[:], start=True, stop=True)
                bjps = pab.tile([128, NODES], f32, tag="bjps")
                nc.tensor.matmul(bjps[:], consts[:, OFF_WA2B : OFF_WA2B + 128], nfr[:], start=True, stop=True)
                bjs = cpool.tile([128, NODES], f32)
                nc.vector.tensor_copy(bjs[:], bjps[:])
                ais = cpool.tile([128, NODES], f32)
                nc.vector.tensor_copy(ais[:], aips[:])
                dps = psh.tile([128, BPC], f32, tag="sh")
                nc.tensor.matmul(dps[:], consts[: SC + 1, OFF_WA2C : OFF_WA2C + 128], sTs[:], start=True, stop=True)
                ai2 = cpool.tile([128, NODES], f32)
                nc.vector.tensor_tensor(
                    ai2[:].rearrange("p (b i) -> p b i", i=NPG),
                    ais[:].rearrange("p (b i) -> p b i", i=NPG),
                    dps[:].unsqueeze(2).broadcast_to([128, BPC, NPG]),
                    op=Alu.add,
                )
                hT = cpool.tile([128, PAIRS], f32)
                nc.vector.tensor_tensor(
                    hT[:].rearrange("p (b i j) -> p b i j", i=NPG, j=NPG),
                    ai2[:].rearrange("p (b i) -> p b i", i=NPG).unsqueeze(3).broadcast_to([128, BPC, NPG, NPG]),
                    bjs[:].rearrange("p (b j) -> p b j", j=NPG).unsqueeze(2).broadcast_to([128, BPC, NPG, NPG]),
                    op=Alu.add,
                )
                nc.vector.tensor_scalar_max(hT[:, : PAIRS // 2], hT[:, : PAIRS // 2], 0.0)
                nc.scalar.activation(hT[:, PAIRS // 2 :], hT[:, PAIRS // 2 :], Act.Relu)

                # ---- saf planes, group-partition layout ----
                # X0[16b+t, pair] = sum_c WfD[c, 16b+t] * hT[c, pair] = saf
                # bond-plane t of every pair (WfD col 16b+t = Wf[:, t]).
                # Rows 16b+3..16b+15 are zero. Then + bf (per-partition col).
                xp1 = ptr.tile([128, PAIRS // 2], f32, tag="tr")
                xp2 = ptr.tile([128, PAIRS // 2], f32, tag="tr")
                nc.tensor.matmul(xp1[:], consts[:, OFF_WFD : OFF_WFD + 128], hT[:, : PAIRS // 2], start=True, stop=True)
                nc.tensor.matmul(xp2[:], consts[:, OFF_WFD : OFF_WFD + 128], hT[:, PAIRS // 2 :], start=True, stop=True)
                x0 = cpool.tile([128, PAIRS], f32)
                nc.vector.tensor_scalar_add(x0[:, : PAIRS // 2], xp1[:], consts[:, OFF_BFC : OFF_BFC + 1])
                nc.scalar.activation(x0[:, PAIRS // 2 :], xp2[:], Act.Identity, bias=consts[:, OFF_BFC : OFF_BFC + 1])

                # ---- gather (pair index per group) + bond select + sum ----
                G3 = cpool.tile([128, 256], f32)
                nc.gpsimd.ap_gather(G3[:], x0[:], idxs[:], channels=128, num_elems=PAIRS, d=1, num_idxs=256)
                P = cpool.tile([128, 256], f32)
                nc.vector.tensor_tensor(P[:], G3[:], selm[:], op=Alu.mult)
                Gp = ptr.tile([BPC, 256], f32, tag="tr")
                nc.tensor.matmul(Gp[:], consts[:, OFF_SUMS : OFF_SUMS + BPC], P[:], start=True, stop=True)
                nmx = cpool.tile([BPC, 1], f32)
                nc.vector.reduce_max(nmx[:], Gp[:, :ASL], axis=mybir.AxisListType.X, negate=True)
                E = cpool.tile([BPC, ASL], f32)
                sums = cpool.tile([BPC, 1], f32)
                nc.scalar.activation(E[:], Gp[:, :ASL], Act.Exp, bias=nmx[:], accum_out=sums[:])
                rc = cpool.tile([BPC, 1], f32)
                nc.vector.reciprocal(rc[:], sums[:])
                OU = cpool.tile([BPC, ASL], f32)
                nc.vector.tensor_scalar_mul(OU[:], E[:], rc[:])
                nc.sync.dma_start(outp_d[:], OU[:])

                if DEBUG_TAPS:
                    taps = {
                        "t_h1bs": h1bs, "t_h1ts": h1ts, "t_sTs": sTs, "t_ro": ro,
                        "t_y1s": y1s, "t_nfr": nfr, "t_ai2": ai2,
                        "t_bjs": bjs, "t_hT": hT, "t_x0": x0,
                        "t_G3": G3, "t_P": P, "t_X2": X2, "t_sums": sums,
                    }
                    for tname, ttile in taps.items():
                        shp = list(ttile[:].shape)
                        td = nc.declare_dram_parameter(tname, shp, f32, isOutput=True)
                        nc.sync.dma_start(td[:], ttile[:])

    nc.compile()
    return nc


def _marshal(node_features, specs, mask, indexmask, W1, b1, W2, b2,
             Wv1, bv1, Wv2, bv2, Wa2, ba2, Wf, bf):
    """Host-side sharding + layout packing. Returns in_maps (one per core)."""
    # W1 with b1 folded as an extra (constant-1-input) row, k-tile packed
    w1p = np.zeros((1802, HIDP), np.float32)
    w1p[:SL, :HID] = W1
    w1p[SL, :HID] = b1
    w1ks = {f"w1k{k}": np.ascontiguousarray(w1p[128 * k : 128 * k + (128 if k < KT - 1 else 10)])
            for k in range(KT)}
    w2p = np.zeros((HIDP, SC), np.float32)
    w2p[:HID] = W2

    consts = np.zeros((128, CF), np.float32)
    consts[:, OFF_WA2A : OFF_WA2A + 128] = Wa2[0:128]
    consts[:, OFF_WA2B : OFF_WA2B + 128] = Wa2[128:256]
    consts[:, OFF_WV1 : OFF_WV1 + 64] = Wv1[0:128]
    consts[:100, OFF_WV1 + 64 : OFF_WV1 + 128] = Wv1[128:228]
    consts[:100, OFF_WA2C : OFF_WA2C + 128] = Wa2[256:356]
    consts[100, OFF_WA2C : OFF_WA2C + 128] = ba2
    consts[:64, OFF_WV2] = Wv2[:, 0]
    consts[:100, OFF_B2] = b2
    consts[:64, OFF_BV1] = bv1
    consts[:1, OFF_BV2] = bv2
    consts[:, OFF_BA2] = ba2
    consts[:BOND, OFF_BF] = bf
    consts[:, OFF_WF : OFF_WF + BOND] = Wf
    consts[:BPC, OFF_EYE : OFF_EYE + BPC] = np.eye(BPC, dtype=np.float32)
    for b in range(BPC):
        consts[:, OFF_WFD + 16 * b : OFF_WFD + 16 * b + BOND] = Wf
        consts[16 * b : 16 * b + BOND, OFF_BFC] = bf
        consts[16 * b + 3, OFF_BFC] = 1.0
        for t in range(BOND):
            consts[16 * b + t, OFF_SUMS + b] = 1.0
    consts[:MCH, OFF_W2T : OFF_W2T + 800] = w2p.reshape(8, MCH, SC).transpose(1, 0, 2).reshape(MCH, 800)

    # index remap: group b gathers pair columns 81*b + v//3 from the plane
    # tile; the bond (v%3) is selected via the SelM one-hot afterwards
    v = indexmask.astype(np.int64)

    in_maps = []
    for c in range(NCORES):
        gsl = slice(c * BPC, (c + 1) * BPC)
        nsl = slice(c * NODES, (c + 1) * NODES)
        # spec transposed + k-tiled + constant-1 bias row (row 1801)
        spc = np.zeros((BPC, KT * 128), np.float32)
        spc[:, :SL] = specs[gsl, 0, :]
        spc[:, SL] = 1.0
        spT = spc.reshape(BPC, KT, 128).transpose(2, 1, 0).reshape(128, KT * BPC)
        acts = np.zeros((128, KT * BPC + NODES), np.float32)
        acts[:, 0 : KT * BPC] = spT
        acts[:, KT * BPC :] = node_features[nsl].T
        vc = v[gsl]                                    # [8, 243]
        padidx = np.zeros((BPC, 256), np.int16)
        padidx[:, :ASL] = (np.arange(BPC)[:, None] * NPG * NPG + vc // BOND).astype(np.int16)
        idx16 = padidx.reshape(BPC, 16, 16).transpose(0, 2, 1).reshape(128, 16)
        selm = np.zeros((128, 256), np.float32)
        for b in range(BPC):
            for t in range(BOND):
                selm[16 * b + t, :ASL] = (vc[b] % BOND == t)
            selm[16 * b + 3, :ASL] = mask[c * BPC + b]
        in_maps.append({
            "consts": consts,
            "acts": acts,
            **w1ks,
            "idx16": np.ascontiguousarray(idx16),
            "selm": selm,
        })
    return in_maps


def _run(inputs, trace=False):
    from concourse.bass_utils import run_bass_kernel_spmd

    if "nc" not in _CACHE:
        _CACHE["nc"] = _build_nc()
    nc = _CACHE["nc"]

    in_maps = _marshal(
        _f32(inputs["node_features"]), _f32(inputs["specs"]),
        _f32(inputs["mask"]), np.asarray(inputs["indexmask"]),
        _f32(inputs["W1"]), _f32(inputs["b1"]), _f32(inputs["W2"]), _f32(inputs["b2"]),
        _f32(inputs["Wv1"]), _f32(inputs["bv1"]), _f32(inputs["Wv2"]), _f32(inputs["bv2"]),
        _f32(inputs["Wa2"]), _f32(inputs["ba2"]), _f32(inputs["Wf"]), _f32(inputs["bf"]),
    )
    res = run_bass_kernel_spmd(nc, in_maps, core_ids=list(range(NCORES)), trace=trace)
    probs = np.concatenate([res.results[c]["out_p"] for c in range(NCORES)], axis=0)
    v = np.concatenate([res.results[c]["out_v"][0] for c in range(NCORES)])[:, None]
    return (probs, v.astype(np.float32)), res


def kernel(**inputs):
    (probs, v), _ = _run(inputs, trace=False)
    return probs, v


# revision 35
# speedup vs baseline: 1.2020x; 1.1612x over previous
"""Trainium2 Bass kernel for nn_ActionPredictionModel (scatter_memory).

Data-parallel over graphs: 8 graphs (72 nodes) per NeuronCore, weights
replicated (collectives measured ~80us launch-skew here, so none used).
Per core:
  - spec MLP layer 1 with W1 as the *moving* operand (b-major output):
    fp32 PE cost is ~2.2ns/col of moving stream, vs ~3.7ns/col as
    stationary, so stationary is the tiny spec tile (8 cols) and all of
    W1 streams through as rhs. b1 is folded in via a constant-1 spec row.
    h1 [8, 904] is then relu'd and PE-transposed (8x [8,113]->[113,8])
    for the hid-major layer 2.
  - value head (sum-pool readout + spec -> scalar)
  - pair action features; block-diagonal structure: only the 9x9
    same-graph pair blocks are materialized ([128ch, 648pairs])
  - per-graph flatten + indexmask gather (gpsimd ap_gather, ucode
    warmed early) + softmax (DVE + one ACT Exp, table warmed early)
Host does only sharding/layout marshalling (transpose, pad, tile-pack,
index remap to the on-device fp layout) and output concatenation.
"""

import numpy as np

# problem dims (hardcoded per contract)
B, NPG, H = 64, 9, 128
SL, SC, BOND, ASL = 1801, 100, 3, 243
NCORES = 8
BPC = B // NCORES            # graphs per core = 8
NODES = BPC * NPG            # nodes per core = 72
PAIRS = BPC * NPG * NPG      # same-graph pairs per core = 648

KT = 15                      # k-tiles over spec dim (14*128 + 10(incl bias row))
HID = 900
HIDP = 904                   # padded hidden (8 * 113)
MCH = HIDP // 8              # hid chunk for transposes / L2 = 113
HHALF = HIDP // 2            # 452 (psum bank-sized moving chunks)

# consts column offsets
OFF_WA2A, OFF_WA2B, OFF_WV1, OFF_WA2C = 0, 128, 256, 384
OFF_WV2, OFF_B2, OFF_BV1, OFF_BV2, OFF_BA2, OFF_BF = 512, 513, 514, 515, 516, 517
OFF_WF, OFF_EYE, OFF_W2T = 518, 521, 529
OFF_WFD, OFF_BFC, OFF_SUMS = 1329, 1457, 1458
CF = 1458 + 128

_CACHE = {}
DEBUG_TAPS = False


def _f32(x):
    return np.ascontiguousarray(np.asarray(x), dtype=np.float32)


def _build_nc():
    import concourse.mybir as mybir
    import concourse.tile as tile
    import concourse.bacc as bacc
    import concourse.bass as bass

    f32 = mybir.dt.float32
    i16 = mybir.dt.int16
    Alu = mybir.AluOpType
    Act = mybir.ActivationFunctionType

    nc = bacc.Bacc("TRN2", target_bir_lowering=False, debug=False, num_devices=1)

    consts_d = nc.declare_dram_parameter("consts", [128, CF], f32, isOutput=False)
    acts_d = nc.declare_dram_parameter("acts", [128, KT * BPC + NODES], f32, isOutput=False)
    w1k_d = [nc.declare_dram_parameter(f"w1k{k}", [128 if k < KT - 1 else 10, HIDP], f32, isOutput=False)
             for k in range(KT)]
    idx_d = nc.declare_dram_parameter("idx16", [128, 16], i16, isOutput=False)
    selm_d = nc.declare_dram_parameter("selm", [128, 256], f32, isOutput=False)
    outp_d = nc.declare_dram_parameter("out_p", [BPC, ASL], f32, isOutput=True)
    outv_d = nc.declare_dram_parameter("out_v", [1, BPC], f32, isOutput=True)

    with tile.TileContext(nc) as tc:
        with (
            tc.tile_pool(name="cpool", bufs=1) as cpool,
            tc.tile_pool(name="pab", bufs=1, space="PSUM") as pab,
            tc.tile_pool(name="psh", bufs=2, space="PSUM") as psh,
            tc.tile_pool(name="ph1", bufs=2, space="PSUM") as ph1,
            tc.tile_pool(name="ptr", bufs=2, space="PSUM") as ptr,
        ):
            # ---- input loads: acts + per-k W1 chunks gate the stream ----
            # (k=14 is tiny -> loads first so the stream starts early)
            acts = cpool.tile([128, KT * BPC + NODES], f32)
            nc.scalar.dma_start(acts[:], acts_d[:])
            w1ts = [None] * KT
            for k in range(KT):
                kk = 128 if k < KT - 1 else 10
                wt = cpool.tile([kk, HIDP], f32, tag=f"w1k{k}")
                eng = nc.sync if k % 2 == 0 else nc.scalar
                eng.dma_start(wt[:], w1k_d[k][:])
                w1ts[k] = wt
            consts = cpool.tile([128, CF], f32)
            nc.sync.dma_start(consts[:], consts_d[:])
            idxs = cpool.tile([128, 16], i16)
            nc.sync.dma_start(idxs[:], idx_d[:])

            # bond-select + mask input (mask rides in selm rows 16b+3)
            selm = cpool.tile([128, 256], f32)
            nc.scalar.dma_start(selm[:], selm_d[:])

            # warm-ups: ACT Exp table + gpsimd ap_gather ucode (hide under stream)
            warm = cpool.tile([1, 1], f32)
            nc.vector.memset(warm[:], 0.0)
            warmo = cpool.tile([1, 1], f32)
            nc.scalar.activation(warmo[:], warm[:], Act.Exp)

            # warm ap_gather: pulls the gpsimd ucode library load off the
            # critical path (the real gather otherwise pays ~8-10us for it)
            gwi = cpool.tile([16, 4], f32)
            gwx = cpool.tile([16, 1], i16)
            gwo = cpool.tile([16, 16], f32)
            nc.vector.memset(gwi[:], 0.0)
            nc.vector.memset(gwx[:], 0)
            nc.gpsimd.ap_gather(gwo[:], gwi[:], gwx[:], channels=16, num_elems=4, d=1, num_idxs=16)

            sp = acts[:, 0 : KT * BPC]               # spT own graphs [128, 15*8]
            nf = acts[:, KT * BPC : KT * BPC + NODES]  # nfT own graphs [128, 72]

            # ---- spec MLP layer 1: b-major, W1 moving, accumulate in PSUM ----
            h1bs = cpool.tile([BPC, HIDP], f32)
            if True:
                h1p0 = ph1.tile([BPC, HHALF], f32, tag="h1b")
                h1p1 = ph1.tile([BPC, HHALF], f32, tag="h1b")
                for k in range(KT):
                    kk = 128 if k < KT - 1 else 10  # last tile: 9 spec rows + bias row
                    wsl = w1ts[k][:]
                    lhs = sp[:kk, BPC * k : BPC * (k + 1)]
                    nc.tensor.matmul(h1p0[:], lhs, wsl[:, :HHALF],
                                     start=(k == 0), stop=(k == KT - 1))
                    nc.tensor.matmul(h1p1[:], lhs, wsl[:, HHALF:],
                                     start=(k == 0), stop=(k == KT - 1))
                # relu (bias already folded via the constant-1 spec row)
                nc.vector.tensor_scalar_max(h1bs[:, :HHALF], h1p0[:], 0.0)
                nc.vector.tensor_scalar_max(h1bs[:, HHALF:], h1p1[:], 0.0)

            if True:
                # ---- transpose h1 to hid-major: 8x PE transpose [8,113]->[113,8] ----
                eye = consts[:BPC, OFF_EYE : OFF_EYE + BPC]
                h1ts = cpool.tile([MCH, 64], f32)
                for j in range(8):
                    tp = ptr.tile([MCH, BPC], f32, tag="tr")
                    nc.tensor.transpose(tp[:], h1bs[:, MCH * j : MCH * (j + 1)], eye)
                    nc.vector.tensor_copy(h1ts[:, BPC * j : BPC * (j + 1)], tp[:])

                # ---- layer 2: sT[q, b] accumulated over 8 hid chunks ----
                sps = psh.tile([SC, BPC], f32, tag="sh")
                for j in range(8):
                    nc.tensor.matmul(
                        sps[:],
                        consts[:MCH, OFF_W2T + SC * j : OFF_W2T + SC * (j + 1)],
                        h1ts[:, BPC * j : BPC * (j + 1)],
                        start=(j == 0), stop=(j == 7),
                    )
                sTs = cpool.tile([SC + 1, BPC], f32)
                nc.vector.memset(sTs[:], 1.0)
                nc.vector.tensor_scalar(sTs[:SC, :], sps[:], consts[:SC, OFF_B2 : OFF_B2 + 1],
                                        0.0, op0=Alu.add, op1=Alu.max)

                # ---- value head ----
                ro = cpool.tile([128, BPC], f32)
                nc.vector.reduce_sum(ro[:], nf.rearrange("p (b n) -> p b n", n=NPG),
                                     axis=mybir.AxisListType.X)
                y1 = psh.tile([64, BPC], f32, tag="sh")
                nc.tensor.matmul(y1[:], consts[:, OFF_WV1 : OFF_WV1 + 64], ro[:], start=True, stop=False)
                nc.tensor.matmul(y1[:], consts[:SC, OFF_WV1 + 64 : OFF_WV1 + 128], sTs[:SC, :], start=False, stop=True)
                y1s = cpool.tile([64, BPC], f32)
                nc.vector.tensor_scalar(y1s[:], y1[:], consts[:64, OFF_BV1 : OFF_BV1 + 1],
                                        0.0, op0=Alu.add, op1=Alu.max)
                vps = psh.tile([1, BPC], f32, tag="sh")
                nc.tensor.matmul(vps[:], consts[:64, OFF_WV2 : OFF_WV2 + 1], y1s[:], start=True, stop=True)
                vs = cpool.tile([1, BPC], f32)
                nc.vector.tensor_scalar_add(vs[:], vps[:], consts[:1, OFF_BV2 : OFF_BV2 + 1])
                nc.scalar.dma_start(outv_d[:], vs[:])

                # ---- pair features: hT[c, (b,i,j)] ----
                nfr = cpool.tile([128, NODES], f32)
                nc.vector.tensor_scalar_max(nfr[:], nf, 0.0)
                aips = pab.tile([128, NODES], f32, tag="aips")
                nc.tensor.matmul(aips[:], consts[:, OFF_WA2A : OFF_WA2A + 128], nfr[:], start=True, stop=True)
                bjps = pab.tile([128, NODES], f32, tag="bjps")
                nc.tensor.matmul(bjps[:], consts[:, OFF_WA2B : OFF_WA2B + 128], nfr[:], start=True, stop=True)
                bjs = cpool.tile([128, NODES], f32)
                nc.vector.tensor_copy(bjs[:], bjps[:])
                ais = cpool.tile([128, NODES], f32)
                nc.vector.tensor_copy(ais[:], aips[:])
                dps = psh.tile([128, BPC], f32, tag="sh")
                nc.tensor.matmul(dps[:], consts[: SC + 1, OFF_WA2C : OFF_WA2C + 128], sTs[:], start=True, stop=True)
                ai2 = cpool.tile([128, NODES], f32)
                nc.vector.tensor_tensor(
                    ai2[:].rearrange("p (b i) -> p b i", i=NPG),
                    ais[:].rearrange("p (b i) -> p b i", i=NPG),
                    dps[:].unsqueeze(2).broadcast_to([128, BPC, NPG]),
                    op=Alu.add,
                )
                hT = cpool.tile([128, PAIRS], f32)
                nc.vector.tensor_tensor(
                    hT[:].rearrange("p (b i j) -> p b i j", i=NPG, j=NPG),
                    ai2[:].rearrange("p (b i) -> p b i", i=NPG).unsqueeze(3).broadcast_to([128, BPC, NPG, NPG]),
                    bjs[:].rearrange("p (b j) -> p b j", j=NPG).unsqueeze(2).broadcast_to([128, BPC, NPG, NPG]),
                    op=Alu.add,
                )
                nc.vector.tensor_scalar_max(hT[:, : PAIRS // 2], hT[:, : PAIRS // 2], 0.0)
                nc.scalar.activation(hT[:, PAIRS // 2 :], hT[:, PAIRS // 2 :], Act.Relu)

                # ---- saf planes, group-partition layout ----
                # X0[16b+t, pair] = sum_c WfD[c, 16b+t] * hT[c, pair] = saf
                # bond-plane t of every pair (WfD col 16b+t = Wf[:, t]).
                # Rows 16b+3..16b+15 are zero. Then + bf (per-partition col).
                xp1 = ptr.tile([128, PAIRS // 2], f32, tag="tr")
                xp2 = ptr.tile([128, PAIRS // 2], f32, tag="tr")
                nc.tensor.matmul(xp1[:], consts[:, OFF_WFD : OFF_WFD + 128], hT[:, : PAIRS // 2], start=True, stop=True)
                nc.tensor.matmul(xp2[:], consts[:, OFF_WFD : OFF_WFD + 128], hT[:, PAIRS // 2 :], start=True, stop=True)
                x0 = cpool.tile([128, PAIRS], f32)
                nc.vector.tensor_scalar_add(x0[:, : PAIRS // 2], xp1[:], consts[:, OFF_BFC : OFF_BFC + 1])
                nc.scalar.activation(x0[:, PAIRS // 2 :], xp2[:], Act.Identity, bias=consts[:, OFF_BFC : OFF_BFC + 1])

                # ---- gather (pair index per group) + bond select + sum ----
                G3 = cpool.tile([128, 256], f32)
                nc.gpsimd.ap_gather(G3[:], x0[:], idxs[:], channels=128, num_elems=PAIRS, d=1, num_idxs=256)
                P = cpool.tile([128, 256], f32)
                nc.vector.tensor_tensor(P[:], G3[:], selm[:], op=Alu.mult)
                Gp = ptr.tile([BPC, 256], f32, tag="tr")
                nc.tensor.matmul(Gp[:], consts[:, OFF_SUMS : OFF_SUMS + BPC], P[:], start=True, stop=True)
                nmx = cpool.tile([BPC, 1], f32)
                nc.vector.reduce_max(nmx[:], Gp[:, :ASL], axis=mybir.AxisListType.X, negate=True)
                E = cpool.tile([BPC, ASL], f32)
                sums = cpool.tile([BPC, 1], f32)
                nc.scalar.activation(E[:], Gp[:, :ASL], Act.Exp, bias=nmx[:], accum_out=sums[:])
                rc = cpool.tile([BPC, 1], f32)
                nc.vector.reciprocal(rc[:], sums[:])
                OU = cpool.tile([BPC, ASL], f32)
                nc.vector.tensor_scalar_mul(OU[:], E[:], rc[:])
                nc.sync.dma_start(outp_d[:], OU[:])
                nc.sync.dma_start(outv_d[:], vs[:])

                if DEBUG_TAPS:
                    taps = {
                        "t_h1bs": h1bs, "t_h1ts": h1ts, "t_sTs": sTs, "t_ro": ro,
                        "t_y1s": y1s, "t_nfr": nfr, "t_ai2": ai2,
                        "t_bjs": bjs, "t_hT": hT, "t_x0": x0,
                        "t_G3": G3, "t_P": P, "t_sums": sums,
                    }
                    for tname, ttile in taps.items():
                        shp = list(ttile[:].shape)
                        td = nc.declare_dram_parameter(tname, shp, f32, isOutput=True)
                        nc.sync.dma_start(td[:], ttile[:])

    nc.compile()
    return nc


def _marshal(node_features, specs, mask, indexmask, W1, b1, W2, b2,
             Wv1, bv1, Wv2, bv2, Wa2, ba2, Wf, bf):
    """Host-side sharding + layout packing. Returns in_maps (one per core)."""
    # W1 with b1 folded as an extra (constant-1-input) row, k-tile packed
    w1p = np.zeros((1802, HIDP), np.float32)
    w1p[:SL, :HID] = W1
    w1p[SL, :HID] = b1
    w1ks = {f"w1k{k}": np.ascontiguousarray(w1p[128 * k : 128 * k + (128 if k < KT - 1 else 10)])
            for k in range(KT)}
    w2p = np.zeros((HIDP, SC), np.float32)
    w2p[:HID] = W2

    consts = np.zeros((128, CF), np.float32)
    consts[:, OFF_WA2A : OFF_WA2A + 128] = Wa2[0:128]
    consts[:, OFF_WA2B : OFF_WA2B + 128] = Wa2[128:256]
    consts[:, OFF_WV1 : OFF_WV1 + 64] = Wv1[0:128]
    consts[:100, OFF_WV1 + 64 : OFF_WV1 + 128] = Wv1[128:228]
    consts[:100, OFF_WA2C : OFF_WA2C + 128] = Wa2[256:356]
    consts[100, OFF_WA2C : OFF_WA2C + 128] = ba2
    consts[:64, OFF_WV2] = Wv2[:, 0]
    consts[:100, OFF_B2] = b2
    consts[:64, OFF_BV1] = bv1
    consts[:1, OFF_BV2] = bv2
    consts[:, OFF_BA2] = ba2
    consts[:BOND, OFF_BF] = bf
    consts[:, OFF_WF : OFF_WF + BOND] = Wf
    consts[:BPC, OFF_EYE : OFF_EYE + BPC] = np.eye(BPC, dtype=np.float32)
    for b in range(BPC):
        consts[:, OFF_WFD + 16 * b : OFF_WFD + 16 * b + BOND] = Wf
        consts[16 * b : 16 * b + BOND, OFF_BFC] = bf
        consts[16 * b + 3, OFF_BFC] = 1.0
        for t in range(BOND):
            consts[16 * b + t, OFF_SUMS + b] = 1.0
    consts[:MCH, OFF_W2T : OFF_W2T + 800] = w2p.reshape(8, MCH, SC).transpose(1, 0, 2).reshape(MCH, 800)

    # index remap: group b gathers pair columns 81*b + v//3 from the plane
    # tile; the bond (v%3) is selected via the SelM one-hot afterwards
    v = indexmask.astype(np.int64)

    in_maps = []
    for c in range(NCORES):
        gsl = slice(c * BPC, (c + 1) * BPC)
        nsl = slice(c * NODES, (c + 1) * NODES)
        # spec transposed + k-tiled + constant-1 bias row (row 1801)
        spc = np.zeros((BPC, KT * 128), np.float32)
        spc[:, :SL] = specs[gsl, 0, :]
        spc[:, SL] = 1.0
        spT = spc.reshape(BPC, KT, 128).transpose(2, 1, 0).reshape(128, KT * BPC)
        acts = np.zeros((128, KT * BPC + NODES), np.float32)
        acts[:, 0 : KT * BPC] = spT
        acts[:, KT * BPC :] = node_features[nsl].T
        vc = v[gsl]                                    # [8, 243]
        padidx = np.zeros((BPC, 256), np.int16)
        padidx[:, :ASL] = (np.arange(BPC)[:, None] * NPG * NPG + vc // BOND).astype(np.int16)
        idx16 = padidx.reshape(BPC, 16, 16).transpose(0, 2, 1).reshape(128, 16)
        selm = np.zeros((128, 256), np.float32)
        for b in range(BPC):
            for t in range(BOND):
                selm[16 * b + t, :ASL] = (vc[b] % BOND == t)
            selm[16 * b + 3, :ASL] = mask[c * BPC + b]
        in_maps.append({
            "consts": consts,
            "acts": acts,
            **w1ks,
            "idx16": np.ascontiguousarray(idx16),
            "selm": selm,
        })
    return in_maps


def _run(inputs, trace=False):
    from concourse.bass_utils import run_bass_kernel_spmd

    if "nc" not in _CACHE:
        _CACHE["nc"] = _build_nc()
    nc = _CACHE["nc"]

    in_maps = _marshal(
        _f32(inputs["node_features"]), _f32(inputs["specs"]),
        _f32(inputs["mask"]), np.asarray(inputs["indexmask"]),
        _f32(inputs["W1"]), _f32(inputs["b1"]), _f32(inputs["W2"]), _f32(inputs["b2"]),
        _f32(inputs["Wv1"]), _f32(inputs["bv1"]), _f32(inputs["Wv2"]), _f32(inputs["bv2"]),
        _f32(inputs["Wa2"]), _f32(inputs["ba2"]), _f32(inputs["Wf"]), _f32(inputs["bf"]),
    )
    res = run_bass_kernel_spmd(nc, in_maps, core_ids=list(range(NCORES)), trace=trace)
    probs = np.concatenate([res.results[c]["out_p"] for c in range(NCORES)], axis=0)
    v = np.concatenate([res.results[c]["out_v"][0] for c in range(NCORES)])[:, None]
    return (probs, v.astype(np.float32)), res


def kernel(**inputs):
    (probs, v), _ = _run(inputs, trace=False)
    return probs, v


# revision 37
# speedup vs baseline: 1.2282x; 1.0218x over previous
"""Trainium2 Bass kernel for nn_ActionPredictionModel (scatter_memory).

Data-parallel over graphs: 8 graphs (72 nodes) per NeuronCore, weights
replicated (collectives measured ~80us launch-skew here, so none used).
Per core:
  - spec MLP layer 1 with W1 as the *moving* operand (b-major output):
    fp32 PE cost is ~2.2ns/col of moving stream, vs ~3.7ns/col as
    stationary, so stationary is the tiny spec tile (8 cols) and all of
    W1 streams through as rhs. b1 is folded in via a constant-1 spec row.
    h1 [8, 904] is then relu'd and PE-transposed (8x [8,113]->[113,8])
    for the hid-major layer 2.
  - value head (sum-pool readout + spec -> scalar)
  - pair action features; block-diagonal structure: only the 9x9
    same-graph pair blocks are materialized ([128ch, 648pairs])
  - per-graph flatten + indexmask gather (gpsimd ap_gather, ucode
    warmed early) + softmax (DVE + one ACT Exp, table warmed early)
Host does only sharding/layout marshalling (transpose, pad, tile-pack,
index remap to the on-device fp layout) and output concatenation.
"""

import numpy as np

# problem dims (hardcoded per contract)
B, NPG, H = 64, 9, 128
SL, SC, BOND, ASL = 1801, 100, 3, 243
NCORES = 8
BPC = B // NCORES            # graphs per core = 8
NODES = BPC * NPG            # nodes per core = 72
PAIRS = BPC * NPG * NPG      # same-graph pairs per core = 648

KT = 15                      # k-tiles over spec dim (14*128 + 10(incl bias row))
HID = 900
HIDP = 904                   # padded hidden (8 * 113)
MCH = HIDP // 8              # hid chunk for transposes / L2 = 113
HHALF = HIDP // 2            # 452 (psum bank-sized moving chunks)

# consts column offsets
OFF_WA2A, OFF_WA2B, OFF_WV1, OFF_WA2C = 0, 128, 256, 384
OFF_WV2, OFF_B2, OFF_BV1, OFF_BV2, OFF_BA2, OFF_BF = 512, 513, 514, 515, 516, 517
OFF_WF, OFF_EYE, OFF_W2T = 518, 521, 529
OFF_WFD, OFF_BFC, OFF_SUMS = 1329, 1457, 1458
CF = 1458 + 128

_CACHE = {}
DEBUG_TAPS = False


def _f32(x):
    return np.ascontiguousarray(np.asarray(x), dtype=np.float32)


def _build_nc():
    import concourse.mybir as mybir
    import concourse.tile as tile
    import concourse.bacc as bacc
    import concourse.bass as bass

    f32 = mybir.dt.float32
    i16 = mybir.dt.int16
    Alu = mybir.AluOpType
    Act = mybir.ActivationFunctionType

    nc = bacc.Bacc("TRN2", target_bir_lowering=False, debug=False, num_devices=1)

    consts_d = nc.declare_dram_parameter("consts", [128, CF], f32, isOutput=False)
    acts_d = nc.declare_dram_parameter("acts", [128, KT * BPC + NODES], f32, isOutput=False)
    w1k_d = [nc.declare_dram_parameter(f"w1k{k}", [128 if k < KT - 1 else 10, HIDP], f32, isOutput=False)
             for k in range(KT)]
    idx_d = nc.declare_dram_parameter("idx16", [128, 16], i16, isOutput=False)
    selm_d = nc.declare_dram_parameter("selm", [128, 256], f32, isOutput=False)
    outp_d = nc.declare_dram_parameter("out_p", [BPC, ASL], f32, isOutput=True)
    outv_d = nc.declare_dram_parameter("out_v", [1, BPC], f32, isOutput=True)

    with tile.TileContext(nc) as tc:
        with (
            tc.tile_pool(name="cpool", bufs=1) as cpool,
            tc.tile_pool(name="pab", bufs=1, space="PSUM") as pab,
            tc.tile_pool(name="psh", bufs=2, space="PSUM") as psh,
            tc.tile_pool(name="ph1", bufs=2, space="PSUM") as ph1,
            tc.tile_pool(name="ptr", bufs=2, space="PSUM") as ptr,
        ):
            # ---- input loads: acts + per-k W1 chunks gate the stream ----
            # (k=14 is tiny -> loads first so the stream starts early)
            acts = cpool.tile([128, KT * BPC + NODES], f32)
            nc.scalar.dma_start(acts[:], acts_d[:])
            w1ts = [None] * KT
            for k in range(KT):
                kk = 128 if k < KT - 1 else 10
                wt = cpool.tile([kk, HIDP], f32, tag=f"w1k{k}")
                eng = nc.sync if k % 2 == 0 else nc.scalar
                eng.dma_start(wt[:], w1k_d[k][:])
                w1ts[k] = wt
            consts = cpool.tile([128, CF], f32)
            nc.sync.dma_start(consts[:], consts_d[:])
            idxs = cpool.tile([128, 16], i16)
            nc.sync.dma_start(idxs[:], idx_d[:])

            # bond-select + mask input (mask rides in selm rows 16b+3)
            selm = cpool.tile([128, 256], f32)
            nc.scalar.dma_start(selm[:], selm_d[:])

            # warm-ups: ACT Exp table + gpsimd ap_gather ucode (hide under stream)
            warm = cpool.tile([1, 1], f32)
            nc.vector.memset(warm[:], 0.0)
            warmo = cpool.tile([1, 1], f32)
            nc.scalar.activation(warmo[:], warm[:], Act.Exp)

            # warm ap_gather: pulls the gpsimd ucode library load off the
            # critical path (the real gather otherwise pays ~8-10us for it)
            gwi = cpool.tile([16, 4], f32)
            gwx = cpool.tile([16, 1], i16)
            gwo = cpool.tile([16, 16], f32)
            nc.vector.memset(gwi[:], 0.0)
            nc.vector.memset(gwx[:], 0)
            nc.gpsimd.ap_gather(gwo[:], gwi[:], gwx[:], channels=16, num_elems=4, d=1, num_idxs=16)

            sp = acts[:, 0 : KT * BPC]               # spT own graphs [128, 15*8]
            nf = acts[:, KT * BPC : KT * BPC + NODES]  # nfT own graphs [128, 72]

            # ---- spec MLP layer 1: b-major, W1 moving, accumulate in PSUM ----
            h1bs = cpool.tile([BPC, HIDP], f32)
            if True:
                h1p0 = ph1.tile([BPC, HHALF], f32, tag="h1b")
                h1p1 = ph1.tile([BPC, HHALF], f32, tag="h1b")
                for k in range(KT):
                    kk = 128 if k < KT - 1 else 10  # last tile: 9 spec rows + bias row
                    wsl = w1ts[k][:]
                    lhs = sp[:kk, BPC * k : BPC * (k + 1)]
                    nc.tensor.matmul(h1p0[:], lhs, wsl[:, :HHALF],
                                     start=(k == 0), stop=(k == KT - 1))
                    nc.tensor.matmul(h1p1[:], lhs, wsl[:, HHALF:],
                                     start=(k == 0), stop=(k == KT - 1))
                # relu (bias already folded via the constant-1 spec row)
                nc.vector.tensor_scalar_max(h1bs[:, :HHALF], h1p0[:], 0.0)
                nc.vector.tensor_scalar_max(h1bs[:, HHALF:], h1p1[:], 0.0)

            if True:
                # ---- transpose h1 to hid-major: 8x PE transpose [8,113]->[113,8] ----
                eye = consts[:BPC, OFF_EYE : OFF_EYE + BPC]
                h1ts = cpool.tile([MCH, 64], f32)
                for j in range(8):
                    tp = ptr.tile([MCH, BPC], f32, tag="tr")
                    nc.tensor.transpose(tp[:], h1bs[:, MCH * j : MCH * (j + 1)], eye)
                    nc.vector.tensor_copy(h1ts[:, BPC * j : BPC * (j + 1)], tp[:])

                # ---- layer 2: sT[q, b] accumulated over 8 hid chunks ----
                sps = psh.tile([SC, BPC], f32, tag="sh")
                for j in range(8):
                    nc.tensor.matmul(
                        sps[:],
                        consts[:MCH, OFF_W2T + SC * j : OFF_W2T + SC * (j + 1)],
                        h1ts[:, BPC * j : BPC * (j + 1)],
                        start=(j == 0), stop=(j == 7),
                    )
                sTs = cpool.tile([SC + 1, BPC], f32)
                nc.vector.memset(sTs[:], 1.0)
                nc.vector.tensor_scalar(sTs[:SC, :], sps[:], consts[:SC, OFF_B2 : OFF_B2 + 1],
                                        0.0, op0=Alu.add, op1=Alu.max)

                # ---- value head ----
                ro = cpool.tile([128, BPC], f32)
                nc.vector.reduce_sum(ro[:], nf.rearrange("p (b n) -> p b n", n=NPG),
                                     axis=mybir.AxisListType.X)
                y1 = psh.tile([64, BPC], f32, tag="sh")
                nc.tensor.matmul(y1[:], consts[:, OFF_WV1 : OFF_WV1 + 64], ro[:], start=True, stop=False)
                nc.tensor.matmul(y1[:], consts[:SC, OFF_WV1 + 64 : OFF_WV1 + 128], sTs[:SC, :], start=False, stop=True)
                y1s = cpool.tile([64, BPC], f32)
                nc.vector.tensor_scalar(y1s[:], y1[:], consts[:64, OFF_BV1 : OFF_BV1 + 1],
                                        0.0, op0=Alu.add, op1=Alu.max)
                vps = psh.tile([1, BPC], f32, tag="sh")
                nc.tensor.matmul(vps[:], consts[:64, OFF_WV2 : OFF_WV2 + 1], y1s[:], start=True, stop=True)
                vs = cpool.tile([1, BPC], f32)
                nc.vector.tensor_scalar_add(vs[:], vps[:], consts[:1, OFF_BV2 : OFF_BV2 + 1])
                nc.scalar.dma_start(outv_d[:], vs[:])

                # ---- pair features: hT[c, (b,i,j)] ----
                nfr = cpool.tile([128, NODES], f32)
                nc.vector.tensor_scalar_max(nfr[:], nf, 0.0)
                aips = pab.tile([128, NODES], f32, tag="aips")
                nc.tensor.matmul(aips[:], consts[:, OFF_WA2A : OFF_WA2A + 128], nfr[:], start=True, stop=True)
                bjps = pab.tile([128, NODES], f32, tag="bjps")
                nc.tensor.matmul(bjps[:], consts[:, OFF_WA2B : OFF_WA2B + 128], nfr[:], start=True, stop=True)
                bjs = cpool.tile([128, NODES], f32)
                nc.vector.tensor_copy(bjs[:], bjps[:])
                ais = cpool.tile([128, NODES], f32)
                nc.vector.tensor_copy(ais[:], aips[:])
                dps = psh.tile([128, BPC], f32, tag="sh")
                nc.tensor.matmul(dps[:], consts[: SC + 1, OFF_WA2C : OFF_WA2C + 128], sTs[:], start=True, stop=True)
                ai2 = cpool.tile([128, NODES], f32)
                nc.vector.tensor_tensor(
                    ai2[:].rearrange("p (b i) -> p b i", i=NPG),
                    ais[:].rearrange("p (b i) -> p b i", i=NPG),
                    dps[:].unsqueeze(2).broadcast_to([128, BPC, NPG]),
                    op=Alu.add,
                )
                hT = cpool.tile([128, PAIRS], f32)
                nc.vector.tensor_tensor(
                    hT[:].rearrange("p (b i j) -> p b i j", i=NPG, j=NPG),
                    ai2[:].rearrange("p (b i) -> p b i", i=NPG).unsqueeze(3).broadcast_to([128, BPC, NPG, NPG]),
                    bjs[:].rearrange("p (b j) -> p b j", j=NPG).unsqueeze(2).broadcast_to([128, BPC, NPG, NPG]),
                    op=Alu.add,
                )
                nc.vector.tensor_scalar_max(hT[:, : PAIRS // 2], hT[:, : PAIRS // 2], 0.0)
                nc.scalar.activation(hT[:, PAIRS // 2 :], hT[:, PAIRS // 2 :], Act.Relu)

                # ---- saf planes, group-partition layout ----
                # X0[16b+t, pair] = sum_c WfD[c, 16b+t] * hT[c, pair] = saf
                # bond-plane t of every pair (WfD col 16b+t = Wf[:, t]).
                # Rows 16b+3..16b+15 are zero. Then + bf (per-partition col).
                xp1 = ptr.tile([128, PAIRS // 2], f32, tag="tr")
                xp2 = ptr.tile([128, PAIRS // 2], f32, tag="tr")
                nc.tensor.matmul(xp1[:], consts[:, OFF_WFD : OFF_WFD + 128], hT[:, : PAIRS // 2], start=True, stop=True)
                nc.tensor.matmul(xp2[:], consts[:, OFF_WFD : OFF_WFD + 128], hT[:, PAIRS // 2 :], start=True, stop=True)
                x0 = cpool.tile([128, PAIRS], f32)
                nc.vector.tensor_scalar_add(x0[:, : PAIRS // 2], xp1[:], consts[:, OFF_BFC : OFF_BFC + 1])
                nc.scalar.activation(x0[:, PAIRS // 2 :], xp2[:], Act.Identity, bias=consts[:, OFF_BFC : OFF_BFC + 1])

                # ---- gather (pair index per group) + bond select + sum ----
                G3 = cpool.tile([128, 256], f32)
                nc.gpsimd.ap_gather(G3[:], x0[:], idxs[:], channels=128, num_elems=PAIRS, d=1, num_idxs=256)
                P = cpool.tile([128, 256], f32)
                nc.vector.tensor_tensor(P[:], G3[:], selm[:], op=Alu.mult)
                Gp = ptr.tile([BPC, 256], f32, tag="tr")
                nc.tensor.matmul(Gp[:], consts[:, OFF_SUMS : OFF_SUMS + BPC], P[:], start=True, stop=True)
                nmx = cpool.tile([BPC, 1], f32)
                nc.vector.reduce_max(nmx[:], Gp[:, :ASL], axis=mybir.AxisListType.X, negate=True)
                E = cpool.tile([BPC, ASL], f32)
                sums = cpool.tile([BPC, 1], f32)
                nc.scalar.activation(E[:], Gp[:, :ASL], Act.Exp, bias=nmx[:], accum_out=sums[:])
                rc = cpool.tile([BPC, 1], f32)
                nc.vector.reciprocal(rc[:], sums[:])
                OU = cpool.tile([BPC, ASL], f32)
                nc.vector.tensor_scalar_mul(OU[:], E[:], rc[:])
                nc.sync.dma_start(outp_d[:], OU[:])
                nc.sync.dma_start(outv_d[:], vs[:])

                if DEBUG_TAPS:
                    taps = {
                        "t_h1bs": h1bs, "t_h1ts": h1ts, "t_sTs": sTs, "t_ro": ro,
                        "t_y1s": y1s, "t_nfr": nfr, "t_ai2": ai2,
                        "t_bjs": bjs, "t_hT": hT, "t_x0": x0,
                        "t_G3": G3, "t_P": P, "t_sums": sums,
                    }
                    for tname, ttile in taps.items():
                        shp = list(ttile[:].shape)
                        td = nc.declare_dram_parameter(tname, shp, f32, isOutput=True)
                        nc.sync.dma_start(td[:], ttile[:])

    nc.compile()
    return nc


def _marshal(node_features, specs, mask, indexmask, W1, b1, W2, b2,
             Wv1, bv1, Wv2, bv2, Wa2, ba2, Wf, bf):
    """Host-side sharding + layout packing. Returns in_maps (one per core)."""
    # W1 with b1 folded as an extra (constant-1-input) row, k-tile packed
    w1p = np.zeros((1802, HIDP), np.float32)
    w1p[:SL, :HID] = W1
    w1p[SL, :HID] = b1
    w1ks = {f"w1k{k}": np.ascontiguousarray(w1p[128 * k : 128 * k + (128 if k < KT - 1 else 10)])
            for k in range(KT)}
    w2p = np.zeros((HIDP, SC), np.float32)
    w2p[:HID] = W2

    consts = np.zeros((128, CF), np.float32)
    consts[:, OFF_WA2A : OFF_WA2A + 128] = Wa2[0:128]
    consts[:, OFF_WA2B : OFF_WA2B + 128] = Wa2[128:256]
    consts[:, OFF_WV1 : OFF_WV1 + 64] = Wv1[0:128]
    consts[:100, OFF_WV1 + 64 : OFF_WV1 + 128] = Wv1[128:228]
    consts[:100, OFF_WA2C : OFF_WA2C + 128] = Wa2[256:356]
    consts[100, OFF_WA2C : OFF_WA2C + 128] = ba2
    consts[:64, OFF_WV2] = Wv2[:, 0]
    consts[:100, OFF_B2] = b2
    consts[:64, OFF_BV1] = bv1
    consts[:1, OFF_BV2] = bv2
    consts[:, OFF_BA2] = ba2
    consts[:BOND, OFF_BF] = bf
    consts[:, OFF_WF : OFF_WF + BOND] = Wf
    consts[:BPC, OFF_EYE : OFF_EYE + BPC] = np.eye(BPC, dtype=np.float32)
    for b in range(BPC):
        consts[:, OFF_WFD + 16 * b : OFF_WFD + 16 * b + BOND] = Wf
        consts[16 * b : 16 * b + BOND, OFF_BFC] = bf
        consts[16 * b + 3, OFF_BFC] = 1.0
        for t in range(BOND):
            consts[16 * b + t, OFF_SUMS + b] = 1.0
    consts[:MCH, OFF_W2T : OFF_W2T + 800] = w2p.reshape(8, MCH, SC).transpose(1, 0, 2).reshape(MCH, 800)

    # index remap: group b gathers pair columns 81*b + v//3 from the plane
    # tile; the bond (v%3) is selected via the SelM one-hot afterwards
    v = indexmask.astype(np.int64)

    in_maps = []
    for c in range(NCORES):
        gsl = slice(c * BPC, (c + 1) * BPC)
        nsl = slice(c * NODES, (c + 1) * NODES)
        # spec transposed + k-tiled + constant-1 bias row (row 1801)
        spc = np.zeros((BPC, KT * 128), np.float32)
        spc[:, :SL] = specs[gsl, 0, :]
        spc[:, SL] = 1.0
        spT = spc.reshape(BPC, KT, 128).transpose(2, 1, 0).reshape(128, KT * BPC)
        acts = np.zeros((128, KT * BPC + NODES), np.float32)
        acts[:, 0 : KT * BPC] = spT
        acts[:, KT * BPC :] = node_features[nsl].T
        vc = v[gsl]                                    # [8, 243]
        padidx = np.zeros((BPC, 256), np.int16)
        padidx[:, :ASL] = (np.arange(BPC)[:, None] * NPG * NPG + vc // BOND).astype(np.int16)
        idx16 = padidx.reshape(BPC, 16, 16).transpose(0, 2, 1).reshape(128, 16)
        selm = np.zeros((128, 256), np.float32)
        for b in range(BPC):
            for t in range(BOND):
                selm[16 * b + t, :ASL] = (vc[b] % BOND == t)
            selm[16 * b + 3, :ASL] = mask[c * BPC + b]
        in_maps.append({
            "consts": consts,
            "acts": acts,
            **w1ks,
            "idx16": np.ascontiguousarray(idx16),
            "selm": selm,
        })
    return in_maps


def _run(inputs, trace=False):
    from concourse.bass_utils import run_bass_kernel_spmd

    if "nc" not in _CACHE:
        _CACHE["nc"] = _build_nc()
    nc = _CACHE["nc"]

    in_maps = _marshal(
        _f32(inputs["node_features"]), _f32(inputs["specs"]),
        _f32(inputs["mask"]), np.asarray(inputs["indexmask"]),
        _f32(inputs["W1"]), _f32(inputs["b1"]), _f32(inputs["W2"]), _f32(inputs["b2"]),
        _f32(inputs["Wv1"]), _f32(inputs["bv1"]), _f32(inputs["Wv2"]), _f32(inputs["bv2"]),
        _f32(inputs["Wa2"]), _f32(inputs["ba2"]), _f32(inputs["Wf"]), _f32(inputs["bf"]),
    )
    res = run_bass_kernel_spmd(nc, in_maps, core_ids=list(range(NCORES)), trace=trace)
    probs = np.concatenate([res.results[c]["out_p"] for c in range(NCORES)], axis=0)
    v = np.concatenate([res.results[c]["out_v"][0] for c in range(NCORES)])[:, None]
    return (probs, v.astype(np.float32)), res


def kernel(**inputs):
    (probs, v), _ = _run(inputs, trace=False)
    return probs, v
